# revision 1
# baseline (speedup 1.0000x reference)
"""Trainium2 Bass kernel for causal self-attention with 2D RoPE.

Sharding: batch x head-group parallel over 8 NeuronCores.
  core c -> batch b = c // 4, heads h0 = (c % 4) * 3 .. h0+2.

Per-core design (3 heads A=h0, B=h1, C=h2):
- QKV projection in bf16 (fp32 PSUM), seq-major, two seq-block
  accumulation chains interleaved so consecutive PE matmuls never target
  the same PSUM tile (hides accumulation write-drain latency).
- 2D RoPE on q/k: single DVE shuffle-multiply (negative-stride output AP
  against a pre-swapped signed sin table) + cos multiply + gpsimd adds;
  head C's rotated output is written twice (for k-tile pairing).
- q/k moved to d-major with DMA-XBAR transposes (no PE transposes).
- Scores: concurrent K=64 matmul pairs via PE row tiling - heads A,B in
  one 128-row pass; head C pairs adjacent k-tiles against its duplicate.
  Scores land in 1024-wide PSUM strips; one exp per strip (two-run APs
  over the causally trimmed regions). Denominators via an appended
  ones-column on V.
- attn @ V with causally trimmed column ranges.
- Normalization at PSUM eviction with a broadcast 1/den operand:
  computed either as exp(-ln(den)) on the ACT engine + bf16 broadcast
  matmul (short chain; used where ACT is draining) or via an XBAR
  s-major round trip + 128-lane reciprocal + DRAM broadcast read.
- Output projection contracts K=128 (A,B packed) + K=64 (C) with no
  per-head scaling. QKV groups are software-pipelined one block ahead
  of attention; outproj of block j runs inside block j+1's window.
  Host sums the 4 partial outputs per batch.
"""

import sys

sys.path.insert(0, "/opt/trn_rl_repo")

import numpy as np
from ml_dtypes import bfloat16

import concourse.bacc as bacc
import concourse.bass as bass
import concourse.mybir as mybir
from concourse import tile
from concourse.bass_utils import run_bass_kernel_spmd

BF = mybir.dt.bfloat16
F32 = mybir.dt.float32
AF = mybir.ActivationFunctionType
ALU = mybir.AluOpType

P = 128          # partitions
DM = 768         # d_model
HD = 64          # head dim
NHC = 3          # heads per core
NCC = DM // P    # contraction chunks (6)
SQT = 512        # q-block size
QKV = 3 * NHC * HD  # 576


def build_program(S=2048, n_devices=8):
    NS = S // P      # seq chunks of 128 (16)
    NQ = S // SQT    # q blocks of 512 (4)
    KPQ = SQT // P   # k-chunks per q-block (4)

    nc = bacc.Bacc(
        "TRN2", target_bir_lowering=False, debug=False, num_devices=n_devices
    )
    XB = 512
    NXB = S // XB
    xt_d = nc.dram_tensor("xt", [NXB, P, NCC, XB], BF, kind="ExternalInput")
    wqkv_d = nc.dram_tensor("wqkv", [P, NCC, QKV], BF, kind="ExternalInput")
    wo2_d = nc.dram_tensor("wo2", [P, DM], BF, kind="ExternalInput")
    woh2_d = nc.dram_tensor("woh2", [HD, DM], BF, kind="ExternalInput")
    cos_d = nc.dram_tensor("cos", [P, S // P, HD], BF, kind="ExternalInput")
    sin_d = nc.dram_tensor("sin", [P, S // P, HD], BF, kind="ExternalInput")
    mask_d = nc.dram_tensor("masks", [P, P], BF, kind="ExternalInput")
    out_d = nc.dram_tensor("outp", [S, DM], BF, kind="ExternalOutput")
    den_dr = nc.dram_tensor("den_scratch", [NHC, NQ, SQT], BF, kind="Internal")

    with tile.TileContext(nc) as tc:
        with (
            tc.tile_pool(name="const", bufs=1) as const,
            tc.tile_pool(name="resid", bufs=1) as resid,
        ):
            # seq-major rope outputs; cols 192:256 duplicate head C
            q_sb = resid.tile([P, NS, 256], BF)
            k_sb = resid.tile([P, NS, 256], BF)
            v_sb = resid.tile([P, NS, NHC, HD + 1], BF)
            # d-major transposed q/k: block 2s = heads A|B, 2s+1 = C|C
            qT = resid.tile([P, 2 * NS, P], BF)
            kT = resid.tile([P, 2 * NS, P], BF)
            ao2 = resid.tile([P, S], BF)      # normalized A|B attn out
            aoh2 = resid.tile([HD, S], BF)    # normalized C attn out
            wqkv_sb = const.tile([P, NCC, QKV], BF)
            wo2_sb = const.tile([P, DM], BF)
            woh2_sb = const.tile([HD, DM], BF)
            cos_sb = const.tile([P, NS, 384], BF)
            sin_sb = const.tile([P, NS, 384], BF)
            cos64_sb = const.tile([P, NS, HD], BF)
            sin64_sb = const.tile([P, NS, HD], BF)
            mask_sb = const.tile([P, P], BF)  # 1 if p <= f else 0
            ones64 = const.tile([1, HD], BF)
            rows16 = resid.tile([P, SQT], BF)  # den rows staging (0/32/64)
            rt = resid.tile([P, P], BF)        # s-major recip staging
            nc.vector.memset(ones64[:], 1.0)
            nc.vector.memset(rows16[:], 1.0)
            nc.vector.memset(rt[:, NHC * KPQ : P], 1.0)
            nc.vector.memset(v_sb[:, :, :, HD], 1.0)  # denominator ones

            nc.sync.dma_start(wqkv_sb[:, 0:3, :], wqkv_d[:, 0:3, :])
            nc.scalar.dma_start(wqkv_sb[:, 3:NCC, :], wqkv_d[:, 3:NCC, :])
            xt_sb = const.tile([P, NCC, S], BF)
            for b in range(NXB - 1, -1, -1):
                bsl = slice(b * XB, (b + 1) * XB)
                nc.sync.dma_start(xt_sb[:, 0:3, bsl], xt_d[b][:, 0:3, :])
                nc.scalar.dma_start(xt_sb[:, 3:NCC, bsl], xt_d[b][:, 3:NCC, :])
            nc.gpsimd.dma_start(cos64_sb[:], cos_d[:])
            nc.gpsimd.dma_start(sin64_sb[:], sin_d[:])
            # replicate the 64-wide rope tables 6x on-chip
            for small, big_t in ((cos64_sb, cos_sb), (sin64_sb, sin_sb)):
                rep = bass.AP(
                    small.tensor, small.offset,
                    [small.ap[0], [HD, NS], [0, 6], [1, HD]],
                )
                nc.vector.tensor_copy(big_t[:], rep)
            nc.gpsimd.dma_start(mask_sb[:], mask_d[:])
            nc.gpsimd.dma_start(wo2_sb[:], wo2_d[:])
            nc.gpsimd.dma_start(woh2_sb[:], woh2_d[:])

            with (
                tc.tile_pool(name="p1t", bufs=4) as tp,
                tc.tile_pool(name="ep", bufs=6) as ep,
                tc.tile_pool(name="rp", bufs=6) as rp,
                tc.tile_pool(name="dsp", bufs=2) as dsp,
                tc.tile_pool(name="accp", bufs=3) as accp,
            ):
                pools = {}

                def rope_and_pack(s, pqkv):
                    qk = pqkv[:, 0:384]
                    cs = cos_sb[:, s, :]
                    sn = sin_sb[:, s, :]
                    qkd = bass.AP(qk.tensor, qk.offset,
                                  [qk.ap[0], [32, 12], [16, 2], [1, 16]])
                    snd = bass.AP(sn.tensor, sn.offset,
                                  [sn.ap[0], [32, 12], [16, 2], [1, 16]])
                    t = tp.tile([P, 384], F32, tag="ropet")
                    # t[swap16(j)] = qk[j] * sin_swapped[j] in one op
                    tsw = bass.AP(
                        t.tensor, t.offset + 16,
                        [t.ap[0], [32, 12], [-16, 2], [1, 16]],
                    )
                    nc.vector.tensor_tensor(tsw, qkd, snd, ALU.mult)
                    t2 = tp.tile([P, 384], F32, tag="ropet2")
                    nc.vector.tensor_tensor(t2[:], qk[:], cs, ALU.mult)
                    # final add on gpsimd, bf16 out; head C written twice
                    nc.gpsimd.tensor_tensor(
                        q_sb[:, s, 0:192], t2[:, 0:192], t[:, 0:192], ALU.add
                    )
                    nc.gpsimd.tensor_tensor(
                        q_sb[:, s, 192:256], t2[:, 128:192], t[:, 128:192],
                        ALU.add,
                    )
                    nc.gpsimd.tensor_tensor(
                        k_sb[:, s, 0:192], t2[:, 192:384], t[:, 192:384],
                        ALU.add,
                    )
                    nc.gpsimd.tensor_tensor(
                        k_sb[:, s, 192:256], t2[:, 320:384], t[:, 320:384],
                        ALU.add,
                    )
                    nc.vector.tensor_copy(
                        v_sb[:, s, :, 0:HD],
                        pqkv[:, 384:QKV].rearrange("p (h x) -> p h x", x=HD),
                    )

                def qkv_group(g, pp):
                    for sp in (2 * g + 1, 2 * g):
                        s0, s1 = 2 * sp, 2 * sp + 1
                        pq0 = pp.tile([P, QKV], F32, tag="pqkv")
                        pq1 = pp.tile([P, QKV], F32, tag="pqkv")
                        x0 = xt_sb[:, :, s0 * P : (s0 + 1) * P]
                        x1 = xt_sb[:, :, s1 * P : (s1 + 1) * P]
                        for c in range(NCC):
                            st, sp_ = (c == 0), (c == NCC - 1)
                            nc.tensor.matmul(
                                pq0[:, 0:512], x0[:, c, :],
                                wqkv_sb[:, c, 0:512], start=st, stop=sp_,
                            )
                            nc.tensor.matmul(
                                pq1[:, 0:512], x1[:, c, :],
                                wqkv_sb[:, c, 0:512], start=st, stop=sp_,
                            )
                            nc.tensor.matmul(
                                pq0[:, 512:QKV], x0[:, c, :],
                                wqkv_sb[:, c, 512:QKV], start=st, stop=sp_,
                            )
                            nc.tensor.matmul(
                                pq1[:, 512:QKV], x1[:, c, :],
                                wqkv_sb[:, c, 512:QKV], start=st, stop=sp_,
                            )
                        rope_and_pack(s0, pq0)
                        rope_and_pack(s1, pq1)
                    nc.sync.dma_start_transpose(
                        qT[:, 8 * g : 8 * g + 8, :],
                        q_sb[:, 4 * g : 4 * g + 4, :],
                    )
                    nc.sync.dma_start_transpose(
                        kT[:, 8 * g : 8 * g + 8, :],
                        k_sb[:, 4 * g : 4 * g + 4, :],
                    )

                def q_mov(phalf, parity, qj, off):
                    """moving q operand: partition half, blocks
                    {8qj+2t+parity}, cols trimmed to [off:512]."""
                    nblk = KPQ - off // P
                    base = qT[
                        phalf * HD : phalf * HD + HD,
                        8 * qj + off // P * 2 + parity,
                        :,
                    ]
                    return bass.AP(
                        base.tensor, base.offset,
                        [base.ap[0], [2 * P, nblk], [1, P]],
                    )

                def attention_block(qj):
                    nki = KPQ * (qj + 1)
                    qsl = slice(qj * SQT, (qj + 1) * SQT)
                    # heads A,B: row-tiled pairs over the same k-tile
                    paA = pools["pap"].tile([HD + 1, SQT], F32, tag="pa")
                    paB = pools["pap"].tile([HD + 1, SQT], F32, tag="pa")
                    ki_order = list(range(KPQ * qj, nki)) + list(
                        range(KPQ * qj - 1, -1, -1)
                    )
                    for ei, ki in enumerate(ki_order):
                        r = ki - KPQ * qj
                        off = max(r, 0) * P
                        strip = pools["big"].tile([P, 1024], F32, tag="big")
                        e = ep.tile([P, 1024], BF, tag="e")
                        for half in range(2):
                            nc.tensor.matmul(
                                strip[:, 512 * half + off : 512 * half + 512],
                                kT[half * HD : half * HD + HD, 2 * ki, :],
                                q_mov(half, 0, qj, off),
                                start=True, stop=True,
                            )
                        if r >= 0:
                            src = bass.AP(
                                strip.tensor, strip[:, off : off + 1].offset,
                                [strip.ap[0], [512, 2], [1, 512 - off]],
                            )
                            dst = bass.AP(
                                e.tensor, e[:, off : off + 1].offset,
                                [e.ap[0], [512, 2], [1, 512 - off]],
                            )
                            nc.scalar.activation(dst, src, AF.Exp, scale=0.125)
                            em = bass.AP(
                                e.tensor, e[:, off : off + 1].offset,
                                [e.ap[0], [512, 2], [1, P]],
                            )
                            mb = bass.AP(
                                mask_sb.tensor, mask_sb.offset,
                                [mask_sb.ap[0], [0, 2], [1, P]],
                            )
                            nc.vector.tensor_tensor(em, em, mb, ALU.mult)
                        else:
                            nc.scalar.activation(
                                e[:], strip[:], AF.Exp, scale=0.125
                            )
                        st, sp = (ei == 0), (ei == nki - 1)
                        nc.tensor.matmul(
                            paA[:, off:SQT], v_sb[:, ki, 0, :],
                            e[:, off:512], start=st, stop=sp,
                        )
                        nc.tensor.matmul(
                            paB[:, off:SQT], v_sb[:, ki, 1, :],
                            e[:, 512 + off : 1024], start=st, stop=sp,
                        )
                    use_lnexp = True
                    if not use_lnexp:
                        for h, pa in ((0, paA), (1, paB)):
                            nc.vector.tensor_copy(
                                rows16[32 * h : 32 * h + 1, :],
                                pa[HD : HD + 1, :],
                            )
                    # head C: adjacent k-tile pairs against duplicated q/k
                    paC = pools["pap"].tile([HD + 1, SQT], F32, tag="pa")
                    kp_order = list(range(2 * qj, nki // 2)) + list(
                        range(2 * qj - 1, -1, -1)
                    )
                    for pi, kp in enumerate(kp_order):
                        kiA, kiB = 2 * kp, 2 * kp + 1
                        rA = kiA - KPQ * qj
                        rB = kiB - KPQ * qj
                        offA = max(rA, 0) * P
                        offB = max(rB, 0) * P
                        strip = pools["big"].tile([P, 1024], F32, tag="big")
                        e = ep.tile([P, 1024], BF, tag="e")
                        for half, ki, off in ((0, kiA, offA), (1, kiB, offB)):
                            nc.tensor.matmul(
                                strip[:, 512 * half + off : 512 * half + 512],
                                kT[half * HD : half * HD + HD, 2 * ki + 1, :],
                                q_mov(half, 1, qj, off),
                                start=True, stop=True,
                            )
                        if rA >= 0:
                            nc.scalar.activation(
                                e[:, offA:512], strip[:, offA:512],
                                AF.Exp, scale=0.125,
                            )
                            nc.scalar.activation(
                                e[:, 512 + offB : 1024],
                                strip[:, 512 + offB : 1024],
                                AF.Exp, scale=0.125,
                            )
                            em = bass.AP(
                                e.tensor, e[:, offA : offA + 1].offset,
                                [e.ap[0], [512 + P, 2], [1, P]],
                            )
                            mb = bass.AP(
                                mask_sb.tensor, mask_sb.offset,
                                [mask_sb.ap[0], [0, 2], [1, P]],
                            )
                            nc.vector.tensor_tensor(em, em, mb, ALU.mult)
                        else:
                            nc.scalar.activation(
                                e[:], strip[:], AF.Exp, scale=0.125
                            )
                        nc.tensor.matmul(
                            paC[:, offA:SQT], v_sb[:, kiA, 2, :],
                            e[:, offA:512], start=(pi == 0), stop=False,
                        )
                        nc.tensor.matmul(
                            paC[:, offB:SQT], v_sb[:, kiB, 2, :],
                            e[:, 512 + offB : 1024], start=False,
                            stop=(pi == nki // 2 - 1),
                        )
                    if not use_lnexp:
                        nc.vector.tensor_copy(
                            rows16[64:65, :], paC[HD : HD + 1, :]
                        )
                        # s-major round trip: XBAR, 128-lane reciprocal,
                        # XBAR back, 12-descriptor DMA to DRAM
                        stt = rp.tile([P, KPQ, P], BF, tag="st")
                        nc.sync.dma_start_transpose(stt[:], rows16[:])
                        rti = bass.AP(
                            stt.tensor, stt.offset,
                            [stt.ap[0], [32, NHC], [P, KPQ]],
                        )
                        rto = bass.AP(
                            rt.tensor, rt.offset,
                            [rt.ap[0], [KPQ, NHC], [1, KPQ]],
                        )
                        with nc.allow_low_precision(reason="bf16 denom"):
                            nc.vector.reciprocal(rto, rti)
                        rt2 = rp.tile([P, 1, P], BF, tag="rt2")
                        nc.sync.dma_start_transpose(rt2[:], rt[:])
                        dbase = den_dr[:, qj, :]
                        dout = bass.AP(
                            dbase.tensor, dbase.offset,
                            [[NQ * SQT, NHC], [P, KPQ], [1, P]],
                        )
                        nc.gpsimd.dma_start(dout, rt2[0 : NHC * KPQ, 0, :])

                    def den_evict():
                        ds = dsp.tile([P, 2 * SQT], BF, tag="ds")
                        for h, pa in ((0, paA), (1, paB), (2, paC)):
                            dsl = ds[(h % 2) * HD : (h % 2) * HD + HD,
                                     (h // 2) * SQT : (h // 2) * SQT + SQT]
                            if use_lnexp:
                                # 1/den = exp(-ln(den)) on ACT, broadcast
                                # with a bf16 matmul
                                lrow = rp.tile([1, SQT], F32, tag="lrow")
                                nc.scalar.activation(
                                    lrow[:], pa[HD : HD + 1, :], AF.Ln
                                )
                                rrow = rp.tile([1, SQT], BF, tag="rrow")
                                nc.scalar.activation(
                                    rrow[:], lrow[:], AF.Exp, scale=-1.0
                                )
                                dbc = pools["dbp"].tile([HD, SQT], F32, tag="dbc")
                                nc.tensor.matmul(
                                    dbc[:], ones64[:], rrow[:],
                                    start=True, stop=True,
                                )
                                nc.vector.tensor_copy(dsl, dbc[:])
                            else:
                                bc = bass.AP(
                                    den_dr.ap().tensor,
                                    (h * NQ + qj) * SQT, [[0, HD], [1, SQT]],
                                )
                                nc.gpsimd.dma_start(dsl, bc)
                        nc.vector.tensor_tensor(
                            ao2[0:HD, qsl], paA[0:HD, :], ds[0:HD, 0:SQT],
                            ALU.mult,
                        )
                        nc.vector.tensor_tensor(
                            ao2[HD:P, qsl], paB[0:HD, :], ds[HD:P, 0:SQT],
                            ALU.mult,
                        )
                        nc.vector.tensor_tensor(
                            aoh2[:, qsl], paC[0:HD, :],
                            ds[0:HD, SQT : 2 * SQT], ALU.mult,
                        )

                    return den_evict

                def outproj_block(qj):
                    for s in range(qj * KPQ, (qj + 1) * KPQ):
                        sl = slice(s * P, (s + 1) * P)
                        po = pools["big"].tile([P, 1024], F32, tag="big")
                        nc.tensor.matmul(
                            po[:, 0:512], ao2[:, sl], wo2_sb[:, 0:512],
                            start=True, stop=False,
                        )
                        nc.tensor.matmul(
                            po[:, 0:512], aoh2[:, sl], woh2_sb[:, 0:512],
                            start=False, stop=True,
                        )
                        nc.tensor.matmul(
                            po[:, 512:DM], ao2[:, sl], wo2_sb[:, 512:DM],
                            start=True, stop=False,
                        )
                        nc.tensor.matmul(
                            po[:, 512:DM], aoh2[:, sl], woh2_sb[:, 512:DM],
                            start=False, stop=True,
                        )
                        acc = accp.tile([P, DM], BF, tag="acc")
                        if qj == 0:
                            # tail block: ACT is idle by now
                            nc.scalar.copy(acc[:], po[:, 0:DM])
                        else:
                            nc.vector.tensor_copy(acc[:], po[:, 0:DM])
                        nc.sync.dma_start(out_d[sl, :], acc[:])

                with tc.tile_pool(name="p1ps", bufs=4, space="PSUM") as pp:
                    for g in range(NQ - 1, -1, -1):
                        qkv_group(g, pp)
                with (
                    tc.tile_pool(name="bigp", bufs=2, space="PSUM") as bigp,
                    tc.tile_pool(name="paps", bufs=3, space="PSUM") as papp,
                    tc.tile_pool(name="dbps", bufs=1, space="PSUM") as dbpp,
                ):
                    pools["big"] = bigp
                    pools["pap"] = papp
                    pools["dbp"] = dbpp
                    prev = None
                    for qj in range(NQ - 1, -1, -1):
                        de = attention_block(qj)
                        if prev is not None:
                            outproj_block(prev)
                        de()
                        prev = qj
                    outproj_block(0)

    nc.compile()
    return nc


_cache = {}
LAST_RESULT = None


def _get_program(S, n_devices):
    key = (S, n_devices)
    if key not in _cache:
        _cache[key] = build_program(S, n_devices)
    return _cache[key]


def _rope_tables(row_ids, col_ids, S):
    inv = 1.0 / (10000.0 ** (np.arange(0, 32, 2, dtype=np.float64) / 32.0))

    def block(ids):
        ang = ids.astype(np.float64)[:, None] * inv[None, :]
        c = np.concatenate([np.cos(ang), np.cos(ang)], -1)
        # pre-swapped signed form: value at source position j equals the
        # signed sin at swap16(j), so rope runs as one linear-in /
        # swapped-out multiply.
        s_ = np.concatenate([np.sin(ang), -np.sin(ang)], -1)
        return c, s_

    cr, sr = block(np.asarray(row_ids))
    cc, sc = block(np.asarray(col_ids))
    cos64 = np.concatenate([cr, cc], -1)
    sin64 = np.concatenate([sr, sc], -1)
    return cos64.astype(bfloat16), sin64.astype(bfloat16)


def _make_masks():
    pp_ = np.arange(P)[:, None]
    ff = np.arange(P)[None, :]
    return (pp_ <= ff).astype(np.float32).astype(bfloat16)


def kernel(x, row_ids, col_ids, Wq, Wk, Wv, Wo):
    x = np.asarray(x)
    B, S, _ = x.shape
    n_cores = 8
    groups = n_cores // B  # head groups per batch (4)
    hpg = NHC

    nc = _get_program(S, n_cores)
    cos_t, sin_t = _rope_tables(row_ids, col_ids, S)
    cos_t = np.ascontiguousarray(cos_t.reshape(S // P, P, -1).transpose(1, 0, 2))
    sin_t = np.ascontiguousarray(sin_t.reshape(S // P, P, -1).transpose(1, 0, 2))
    masks = _make_masks()

    Wq, Wk, Wv, Wo = (np.asarray(w, np.float32) for w in (Wq, Wk, Wv, Wo))
    in_maps = []
    for c in range(n_cores):
        b = c // groups
        h0 = (c % groups) * hpg
        rows = slice(h0 * HD, (h0 + hpg) * HD)
        xt = np.ascontiguousarray(x[b].T).astype(bfloat16)
        NXB = S // 512
        xt = np.ascontiguousarray(
            xt.reshape(NCC, P, NXB, 512).transpose(2, 1, 0, 3)
        )
        wqkv = np.concatenate(
            [Wq[rows].T, Wk[rows].T, Wv[rows].T], axis=1
        ).astype(bfloat16)
        wqkv = np.ascontiguousarray(wqkv.reshape(NCC, P, QKV).transpose(1, 0, 2))
        wog = np.ascontiguousarray(Wo[:, rows].T).astype(bfloat16)  # [192,768]
        in_maps.append(
            {
                "xt": xt,
                "wqkv": wqkv,
                "wo2": np.ascontiguousarray(wog[0:P]),
                "woh2": np.ascontiguousarray(wog[P : P + HD]),
                "cos": cos_t,
                "sin": sin_t,
                "masks": masks,
            }
        )

    import os

    trace = bool(os.environ.get("KERNEL_TRACE"))
    kw = {}
    if trace and os.environ.get("KERNEL_TRACE_DIR"):
        kw["tmpdir"] = os.environ["KERNEL_TRACE_DIR"]
    res = run_bass_kernel_spmd(nc, in_maps, list(range(n_cores)), trace=trace, **kw)
    global LAST_RESULT
    LAST_RESULT = res

    outs = [res.results[c]["outp"].astype(np.float32) for c in range(n_cores)]
    out = np.stack(
        [sum(outs[b * groups + g] for g in range(groups)) for b in range(B)],
        axis=0,
    )
    return out.astype(np.float32)



# revision 28
# speedup vs baseline: 1.1982x; 1.1982x over previous
"""Trainium2 Bass kernel for causal self-attention with 2D RoPE.

Sharding: batch x head-group parallel over 8 NeuronCores.
  core c -> batch b = c // 4, heads h0 = (c % 4) * 3 .. h0+2.

Per-core design (3 heads A=h0, B=h1, C=h2):
- QKV projection in bf16 (fp32 PSUM), seq-major, two seq-block
  accumulation chains interleaved so consecutive PE matmuls never target
  the same PSUM tile (hides accumulation write-drain latency).
- 2D RoPE on q/k: single DVE shuffle-multiply (negative-stride output AP
  against a pre-swapped signed sin table) + cos multiply + gpsimd adds;
  head C's rotated output is written twice (for k-tile pairing).
- q/k moved to d-major with DMA-XBAR transposes (no PE transposes).
- Scores: concurrent K=64 matmul pairs via PE row tiling - heads A,B in
  one 128-row pass; head C pairs adjacent k-tiles against its duplicate.
  Scores land in 1024-wide PSUM strips; one exp per strip (two-run APs
  over the causally trimmed regions). Denominators via an appended
  ones-column on V.
- attn @ V with causally trimmed column ranges.
- Normalization at PSUM eviction with a broadcast 1/den operand:
  computed either as exp(-ln(den)) on the ACT engine + bf16 broadcast
  matmul (short chain; used where ACT is draining) or via an XBAR
  s-major round trip + 128-lane reciprocal + DRAM broadcast read.
- Output projection contracts K=128 (A,B packed) + K=64 (C) with no
  per-head scaling. QKV groups are software-pipelined one block ahead
  of attention; outproj of block j runs inside block j+1's window.
  Host sums the 4 partial outputs per batch.
"""

import sys

sys.path.insert(0, "/opt/trn_rl_repo")

import numpy as np
from ml_dtypes import bfloat16

import concourse.bacc as bacc
import concourse.bass as bass
import concourse.mybir as mybir
import concourse.hw_specs as _hw_specs

_orig_get_tables = _hw_specs.get_activation_tables.__wrapped__

_COMBO = "natural_log_exp_and_others"
_PIN = {"exp", "ln", "copy", "identity"}


def _tables_pin_combo(module_arch):
    # Keep set order/indices (walrus maps set_id -> act_info.json index);
    # strip exp/ln/copy from all other sets so the table-load chooser
    # resolves them to the combined set -> one ACT_TABLE_LOAD total.
    tabs = _orig_get_tables(module_arch)
    if _COMBO not in tabs:
        return tabs
    pin_fns = {f for f in tabs[_COMBO]
               if getattr(f, "name", str(f)).lower() in _PIN
               or str(f).split(".")[-1].lower() in _PIN}
    out = {}
    for name, fns in tabs.items():
        if name == _COMBO:
            out[name] = fns
        else:
            out[name] = fns - pin_fns
    return out


import functools

_patched = functools.cache(_tables_pin_combo)
_hw_specs.get_activation_tables = _patched
bacc.get_activation_tables = _patched
from concourse import tile
from concourse.bass_utils import run_bass_kernel_spmd

BF = mybir.dt.bfloat16
F32 = mybir.dt.float32
AF = mybir.ActivationFunctionType
ALU = mybir.AluOpType

P = 128          # partitions
DM = 768         # d_model
HD = 64          # head dim
NHC = 3          # heads per core
NCC = DM // P    # contraction chunks (6)
SQT = 512        # q-block size
QKV = 3 * NHC * HD  # 576


def build_program(S=2048, n_devices=8):
    NS = S // P      # seq chunks of 128 (16)
    NQ = S // SQT    # q blocks of 512 (4)
    KPQ = SQT // P   # k-chunks per q-block (4)

    nc = bacc.Bacc(
        "TRN2", target_bir_lowering=False, debug=False, num_devices=n_devices
    )
    XB = 512
    NXB = S // XB
    xt_d = nc.dram_tensor("xt", [NXB, P, NCC, XB], BF, kind="ExternalInput")
    wqkv_d = nc.dram_tensor("wqkv", [P, NCC, QKV], BF, kind="ExternalInput")
    wo2_d = nc.dram_tensor("wo2", [P, DM], BF, kind="ExternalInput")
    woh2_d = nc.dram_tensor("woh2", [HD, DM], BF, kind="ExternalInput")
    cos_d = nc.dram_tensor("cos", [P, S // P, HD], BF, kind="ExternalInput")
    sin_d = nc.dram_tensor("sin", [P, S // P, HD], BF, kind="ExternalInput")
    mask_d = nc.dram_tensor("masks", [P, P], BF, kind="ExternalInput")
    out_d = nc.dram_tensor("outp", [S, DM], BF, kind="ExternalOutput")
    den_dr = nc.dram_tensor("den_scratch", [NHC, NQ, SQT], BF, kind="Internal")

    with tile.TileContext(nc) as tc:
        with (
            tc.tile_pool(name="const", bufs=1) as const,
            tc.tile_pool(name="resid", bufs=1) as resid,
        ):
            # seq-major rope outputs; cols 192:256 duplicate head C
            q_sb = resid.tile([P, NS, 256], BF)
            k_sb = resid.tile([P, NS, 256], BF)
            v_sb = resid.tile([P, NS, NHC, HD + 1], BF)
            # d-major transposed q/k: block 2s = heads A|B, 2s+1 = C|C
            qT = resid.tile([P, 2 * NS, P], BF)
            kT = resid.tile([P, 2 * NS, P], BF)
            ao2 = resid.tile([P, S], BF)      # normalized A|B attn out
            aoh2 = resid.tile([HD, S], BF)    # normalized C attn out
            wqkv_sb = const.tile([P, NCC, QKV], BF)
            wo2_sb = const.tile([P, DM], BF)
            woh2_sb = const.tile([HD, DM], BF)
            cos_sb = const.tile([P, NS, 384], BF)
            sin_sb = const.tile([P, NS, 384], BF)
            cos64_sb = const.tile([P, NS, HD], BF)
            sin64_sb = const.tile([P, NS, HD], BF)
            mask_sb = const.tile([P, P], BF)  # 1 if p <= f else 0
            ones64 = const.tile([1, HD], BF)
            rows16 = resid.tile([P, SQT], BF)  # den rows staging (0/32/64)
            rt = resid.tile([P, P], BF)        # s-major recip staging
            nc.vector.memset(ones64[:], 1.0)
            nc.vector.memset(rows16[:], 1.0)
            nc.vector.memset(rt[:, NHC * KPQ : P], 1.0)
            nc.vector.memset(v_sb[:, :, :, HD], 1.0)  # denominator ones

            nc.sync.dma_start(wqkv_sb[:, 0:3, :], wqkv_d[:, 0:3, :])
            nc.scalar.dma_start(wqkv_sb[:, 3:NCC, :], wqkv_d[:, 3:NCC, :])
            xt_sb = const.tile([P, NCC, S], BF)
            for b in range(NXB - 1, -1, -1):
                bsl = slice(b * XB, (b + 1) * XB)
                nc.sync.dma_start(xt_sb[:, 0:3, bsl], xt_d[b][:, 0:3, :])
                nc.scalar.dma_start(xt_sb[:, 3:NCC, bsl], xt_d[b][:, 3:NCC, :])
            nc.gpsimd.dma_start(cos64_sb[:], cos_d[:])
            nc.gpsimd.dma_start(sin64_sb[:], sin_d[:])
            # replicate the 64-wide rope tables 6x on-chip
            for small, big_t in ((cos64_sb, cos_sb), (sin64_sb, sin_sb)):
                rep = bass.AP(
                    small.tensor, small.offset,
                    [small.ap[0], [HD, NS], [0, 6], [1, HD]],
                )
                nc.vector.tensor_copy(big_t[:], rep)
            nc.gpsimd.dma_start(mask_sb[:], mask_d[:])
            nc.gpsimd.dma_start(wo2_sb[:], wo2_d[:])
            nc.gpsimd.dma_start(woh2_sb[:], woh2_d[:])

            with (
                tc.tile_pool(name="p1t", bufs=4) as tp,
                tc.tile_pool(name="ep", bufs=6) as ep,
                tc.tile_pool(name="rp", bufs=6) as rp,
                tc.tile_pool(name="dsp", bufs=2) as dsp,
                tc.tile_pool(name="accp", bufs=3) as accp,
            ):
                pools = {}

                def rope_and_pack(s, pqkv):
                    qk = pqkv[:, 0:384]
                    cs = cos_sb[:, s, :]
                    sn = sin_sb[:, s, :]
                    qkd = bass.AP(qk.tensor, qk.offset,
                                  [qk.ap[0], [32, 12], [16, 2], [1, 16]])
                    snd = bass.AP(sn.tensor, sn.offset,
                                  [sn.ap[0], [32, 12], [16, 2], [1, 16]])
                    t = tp.tile([P, 384], F32, tag="ropet")
                    # t[swap16(j)] = qk[j] * sin_swapped[j] in one op
                    tsw = bass.AP(
                        t.tensor, t.offset + 16,
                        [t.ap[0], [32, 12], [-16, 2], [1, 16]],
                    )
                    nc.vector.tensor_tensor(tsw, qkd, snd, ALU.mult)
                    t2 = tp.tile([P, 384], F32, tag="ropet2")
                    nc.vector.tensor_tensor(t2[:], qk[:], cs, ALU.mult)
                    # final add on gpsimd, bf16 out; head C written twice
                    nc.gpsimd.tensor_tensor(
                        q_sb[:, s, 0:192], t2[:, 0:192], t[:, 0:192], ALU.add
                    )
                    nc.gpsimd.tensor_tensor(
                        q_sb[:, s, 192:256], t2[:, 128:192], t[:, 128:192],
                        ALU.add,
                    )
                    nc.gpsimd.tensor_tensor(
                        k_sb[:, s, 0:192], t2[:, 192:384], t[:, 192:384],
                        ALU.add,
                    )
                    nc.gpsimd.tensor_tensor(
                        k_sb[:, s, 192:256], t2[:, 320:384], t[:, 320:384],
                        ALU.add,
                    )
                    nc.vector.tensor_copy(
                        v_sb[:, s, :, 0:HD],
                        pqkv[:, 384:QKV].rearrange("p (h x) -> p h x", x=HD),
                    )

                def qkv_group(g, pp):
                    for sp in (2 * g + 1, 2 * g):
                        s0, s1 = 2 * sp, 2 * sp + 1
                        pq0 = pp.tile([P, QKV], F32, tag="pqkv")
                        pq1 = pp.tile([P, QKV], F32, tag="pqkv")
                        x0 = xt_sb[:, :, s0 * P : (s0 + 1) * P]
                        x1 = xt_sb[:, :, s1 * P : (s1 + 1) * P]
                        for c in range(NCC):
                            st, sp_ = (c == 0), (c == NCC - 1)
                            nc.tensor.matmul(
                                pq0[:, 0:512], x0[:, c, :],
                                wqkv_sb[:, c, 0:512], start=st, stop=sp_,
                            )
                            nc.tensor.matmul(
                                pq1[:, 0:512], x1[:, c, :],
                                wqkv_sb[:, c, 0:512], start=st, stop=sp_,
                            )
                            nc.tensor.matmul(
                                pq0[:, 512:QKV], x0[:, c, :],
                                wqkv_sb[:, c, 512:QKV], start=st, stop=sp_,
                            )
                            nc.tensor.matmul(
                                pq1[:, 512:QKV], x1[:, c, :],
                                wqkv_sb[:, c, 512:QKV], start=st, stop=sp_,
                            )
                        rope_and_pack(s0, pq0)
                        rope_and_pack(s1, pq1)
                    nc.sync.dma_start_transpose(
                        qT[:, 8 * g : 8 * g + 8, :],
                        q_sb[:, 4 * g : 4 * g + 4, :],
                    )
                    nc.sync.dma_start_transpose(
                        kT[:, 8 * g : 8 * g + 8, :],
                        k_sb[:, 4 * g : 4 * g + 4, :],
                    )

                def q_mov(phalf, parity, qj, off):
                    """moving q operand: partition half, blocks
                    {8qj+2t+parity}, cols trimmed to [off:512]."""
                    nblk = KPQ - off // P
                    base = qT[
                        phalf * HD : phalf * HD + HD,
                        8 * qj + off // P * 2 + parity,
                        :,
                    ]
                    return bass.AP(
                        base.tensor, base.offset,
                        [base.ap[0], [2 * P, nblk], [1, P]],
                    )

                def attention_block(qj):
                    nki = KPQ * (qj + 1)
                    qsl = slice(qj * SQT, (qj + 1) * SQT)
                    # heads A,B: row-tiled pairs over the same k-tile
                    paA = pools["pap"].tile([HD + 1, SQT], F32, tag="pa")
                    paB = pools["pap"].tile([HD + 1, SQT], F32, tag="pa")
                    ki_order = list(range(KPQ * qj, nki)) + list(
                        range(KPQ * qj - 1, -1, -1)
                    )
                    for ei, ki in enumerate(ki_order):
                        r = ki - KPQ * qj
                        off = max(r, 0) * P
                        strip = pools["big"].tile([P, 1024], F32, tag="big")
                        e = ep.tile([P, 1024], BF, tag="e")
                        for half in range(2):
                            nc.tensor.matmul(
                                strip[:, 512 * half + off : 512 * half + 512],
                                kT[half * HD : half * HD + HD, 2 * ki, :],
                                q_mov(half, 0, qj, off),
                                start=True, stop=True,
                            )
                        if r >= 0:
                            src = bass.AP(
                                strip.tensor, strip[:, off : off + 1].offset,
                                [strip.ap[0], [512, 2], [1, 512 - off]],
                            )
                            dst = bass.AP(
                                e.tensor, e[:, off : off + 1].offset,
                                [e.ap[0], [512, 2], [1, 512 - off]],
                            )
                            nc.scalar.activation(dst, src, AF.Exp, scale=0.125)
                            em = bass.AP(
                                e.tensor, e[:, off : off + 1].offset,
                                [e.ap[0], [512, 2], [1, P]],
                            )
                            mb = bass.AP(
                                mask_sb.tensor, mask_sb.offset,
                                [mask_sb.ap[0], [0, 2], [1, P]],
                            )
                            nc.vector.tensor_tensor(em, em, mb, ALU.mult)
                        else:
                            nc.scalar.activation(
                                e[:], strip[:], AF.Exp, scale=0.125
                            )
                        st, sp = (ei == 0), (ei == nki - 1)
                        nc.tensor.matmul(
                            paA[:, off:SQT], v_sb[:, ki, 0, :],
                            e[:, off:512], start=st, stop=sp,
                        )
                        nc.tensor.matmul(
                            paB[:, off:SQT], v_sb[:, ki, 1, :],
                            e[:, 512 + off : 1024], start=st, stop=sp,
                        )
                    use_lnexp = True
                    if not use_lnexp:
                        for h, pa in ((0, paA), (1, paB)):
                            nc.vector.tensor_copy(
                                rows16[32 * h : 32 * h + 1, :],
                                pa[HD : HD + 1, :],
                            )
                    # head C: adjacent k-tile pairs against duplicated q/k
                    paC = pools["pap"].tile([HD + 1, SQT], F32, tag="pa")
                    kp_order = list(range(2 * qj, nki // 2)) + list(
                        range(2 * qj - 1, -1, -1)
                    )
                    for pi, kp in enumerate(kp_order):
                        kiA, kiB = 2 * kp, 2 * kp + 1
                        rA = kiA - KPQ * qj
                        rB = kiB - KPQ * qj
                        offA = max(rA, 0) * P
                        offB = max(rB, 0) * P
                        strip = pools["big"].tile([P, 1024], F32, tag="big")
                        e = ep.tile([P, 1024], BF, tag="e")
                        for half, ki, off in ((0, kiA, offA), (1, kiB, offB)):
                            nc.tensor.matmul(
                                strip[:, 512 * half + off : 512 * half + 512],
                                kT[half * HD : half * HD + HD, 2 * ki + 1, :],
                                q_mov(half, 1, qj, off),
                                start=True, stop=True,
                            )
                        if rA >= 0:
                            nc.scalar.activation(
                                e[:, offA:512], strip[:, offA:512],
                                AF.Exp, scale=0.125,
                            )
                            nc.scalar.activation(
                                e[:, 512 + offB : 1024],
                                strip[:, 512 + offB : 1024],
                                AF.Exp, scale=0.125,
                            )
                            em = bass.AP(
                                e.tensor, e[:, offA : offA + 1].offset,
                                [e.ap[0], [512 + P, 2], [1, P]],
                            )
                            mb = bass.AP(
                                mask_sb.tensor, mask_sb.offset,
                                [mask_sb.ap[0], [0, 2], [1, P]],
                            )
                            nc.vector.tensor_tensor(em, em, mb, ALU.mult)
                        else:
                            nc.scalar.activation(
                                e[:], strip[:], AF.Exp, scale=0.125
                            )
                        nc.tensor.matmul(
                            paC[:, offA:SQT], v_sb[:, kiA, 2, :],
                            e[:, offA:512], start=(pi == 0), stop=False,
                        )
                        nc.tensor.matmul(
                            paC[:, offB:SQT], v_sb[:, kiB, 2, :],
                            e[:, 512 + offB : 1024], start=False,
                            stop=(pi == nki // 2 - 1),
                        )
                    if not use_lnexp:
                        nc.vector.tensor_copy(
                            rows16[64:65, :], paC[HD : HD + 1, :]
                        )
                        # s-major round trip: XBAR, 128-lane reciprocal,
                        # XBAR back, 12-descriptor DMA to DRAM
                        stt = rp.tile([P, KPQ, P], BF, tag="st")
                        nc.sync.dma_start_transpose(stt[:], rows16[:])
                        rti = bass.AP(
                            stt.tensor, stt.offset,
                            [stt.ap[0], [32, NHC], [P, KPQ]],
                        )
                        rto = bass.AP(
                            rt.tensor, rt.offset,
                            [rt.ap[0], [KPQ, NHC], [1, KPQ]],
                        )
                        with nc.allow_low_precision(reason="bf16 denom"):
                            nc.vector.reciprocal(rto, rti)
                        rt2 = rp.tile([P, 1, P], BF, tag="rt2")
                        nc.sync.dma_start_transpose(rt2[:], rt[:])
                        dbase = den_dr[:, qj, :]
                        dout = bass.AP(
                            dbase.tensor, dbase.offset,
                            [[NQ * SQT, NHC], [P, KPQ], [1, P]],
                        )
                        nc.gpsimd.dma_start(dout, rt2[0 : NHC * KPQ, 0, :])

                    def den_evict():
                        ds = dsp.tile([P, 2 * SQT], BF, tag="ds")
                        for h, pa in ((0, paA), (1, paB), (2, paC)):
                            dsl = ds[(h % 2) * HD : (h % 2) * HD + HD,
                                     (h // 2) * SQT : (h // 2) * SQT + SQT]
                            if use_lnexp:
                                # 1/den = exp(-ln(den)) on ACT, broadcast
                                # with a bf16 matmul
                                lrow = rp.tile([1, SQT], F32, tag="lrow")
                                nc.scalar.activation(
                                    lrow[:], pa[HD : HD + 1, :], AF.Ln
                                )
                                rrow = rp.tile([1, SQT], BF, tag="rrow")
                                nc.scalar.activation(
                                    rrow[:], lrow[:], AF.Exp, scale=-1.0
                                )
                                dbc = pools["dbp"].tile([HD, SQT], F32, tag="dbc")
                                nc.tensor.matmul(
                                    dbc[:], ones64[:], rrow[:],
                                    start=True, stop=True,
                                )
                                nc.vector.tensor_copy(dsl, dbc[:])
                            else:
                                bc = bass.AP(
                                    den_dr.ap().tensor,
                                    (h * NQ + qj) * SQT, [[0, HD], [1, SQT]],
                                )
                                nc.gpsimd.dma_start(dsl, bc)
                        nc.vector.tensor_tensor(
                            ao2[0:HD, qsl], paA[0:HD, :], ds[0:HD, 0:SQT],
                            ALU.mult,
                        )
                        nc.vector.tensor_tensor(
                            ao2[HD:P, qsl], paB[0:HD, :], ds[HD:P, 0:SQT],
                            ALU.mult,
                        )
                        nc.vector.tensor_tensor(
                            aoh2[:, qsl], paC[0:HD, :],
                            ds[0:HD, SQT : 2 * SQT], ALU.mult,
                        )

                    return den_evict

                def outproj_block(qj):
                    for s in range(qj * KPQ, (qj + 1) * KPQ):
                        sl = slice(s * P, (s + 1) * P)
                        po = pools["big"].tile([P, 1024], F32, tag="big")
                        nc.tensor.matmul(
                            po[:, 0:512], ao2[:, sl], wo2_sb[:, 0:512],
                            start=True, stop=False,
                        )
                        nc.tensor.matmul(
                            po[:, 0:512], aoh2[:, sl], woh2_sb[:, 0:512],
                            start=False, stop=True,
                        )
                        nc.tensor.matmul(
                            po[:, 512:DM], ao2[:, sl], wo2_sb[:, 512:DM],
                            start=True, stop=False,
                        )
                        nc.tensor.matmul(
                            po[:, 512:DM], aoh2[:, sl], woh2_sb[:, 512:DM],
                            start=False, stop=True,
                        )
                        acc = accp.tile([P, DM], BF, tag="acc")
                        if qj == 0:
                            # tail block: ACT is idle by now
                            nc.scalar.copy(acc[:], po[:, 0:DM])
                        else:
                            nc.vector.tensor_copy(acc[:], po[:, 0:DM])
                        nc.sync.dma_start(out_d[sl, :], acc[:])

                with tc.tile_pool(name="p1ps", bufs=4, space="PSUM") as pp:
                    for g in range(NQ - 1, -1, -1):
                        qkv_group(g, pp)
                with (
                    tc.tile_pool(name="bigp", bufs=2, space="PSUM") as bigp,
                    tc.tile_pool(name="paps", bufs=3, space="PSUM") as papp,
                    tc.tile_pool(name="dbps", bufs=1, space="PSUM") as dbpp,
                ):
                    pools["big"] = bigp
                    pools["pap"] = papp
                    pools["dbp"] = dbpp
                    prev = None
                    for qj in range(NQ - 1, -1, -1):
                        de = attention_block(qj)
                        if prev is not None:
                            outproj_block(prev)
                        de()
                        prev = qj
                    outproj_block(0)

    nc.compile()
    return nc


_cache = {}
LAST_RESULT = None


def _get_program(S, n_devices):
    key = (S, n_devices)
    if key not in _cache:
        _cache[key] = build_program(S, n_devices)
    return _cache[key]


def _rope_tables(row_ids, col_ids, S):
    inv = 1.0 / (10000.0 ** (np.arange(0, 32, 2, dtype=np.float64) / 32.0))

    def block(ids):
        ang = ids.astype(np.float64)[:, None] * inv[None, :]
        c = np.concatenate([np.cos(ang), np.cos(ang)], -1)
        # pre-swapped signed form: value at source position j equals the
        # signed sin at swap16(j), so rope runs as one linear-in /
        # swapped-out multiply.
        s_ = np.concatenate([np.sin(ang), -np.sin(ang)], -1)
        return c, s_

    cr, sr = block(np.asarray(row_ids))
    cc, sc = block(np.asarray(col_ids))
    cos64 = np.concatenate([cr, cc], -1)
    sin64 = np.concatenate([sr, sc], -1)
    return cos64.astype(bfloat16), sin64.astype(bfloat16)


def _make_masks():
    pp_ = np.arange(P)[:, None]
    ff = np.arange(P)[None, :]
    return (pp_ <= ff).astype(np.float32).astype(bfloat16)


def kernel(x, row_ids, col_ids, Wq, Wk, Wv, Wo):
    x = np.asarray(x)
    B, S, _ = x.shape
    n_cores = 8
    groups = n_cores // B  # head groups per batch (4)
    hpg = NHC

    nc = _get_program(S, n_cores)
    cos_t, sin_t = _rope_tables(row_ids, col_ids, S)
    cos_t = np.ascontiguousarray(cos_t.reshape(S // P, P, -1).transpose(1, 0, 2))
    sin_t = np.ascontiguousarray(sin_t.reshape(S // P, P, -1).transpose(1, 0, 2))
    masks = _make_masks()

    Wq, Wk, Wv, Wo = (np.asarray(w, np.float32) for w in (Wq, Wk, Wv, Wo))
    in_maps = []
    for c in range(n_cores):
        b = c // groups
        h0 = (c % groups) * hpg
        rows = slice(h0 * HD, (h0 + hpg) * HD)
        xt = np.ascontiguousarray(x[b].T).astype(bfloat16)
        NXB = S // 512
        xt = np.ascontiguousarray(
            xt.reshape(NCC, P, NXB, 512).transpose(2, 1, 0, 3)
        )
        wqkv = np.concatenate(
            [Wq[rows].T, Wk[rows].T, Wv[rows].T], axis=1
        ).astype(bfloat16)
        wqkv = np.ascontiguousarray(wqkv.reshape(NCC, P, QKV).transpose(1, 0, 2))
        wog = np.ascontiguousarray(Wo[:, rows].T).astype(bfloat16)  # [192,768]
        in_maps.append(
            {
                "xt": xt,
                "wqkv": wqkv,
                "wo2": np.ascontiguousarray(wog[0:P]),
                "woh2": np.ascontiguousarray(wog[P : P + HD]),
                "cos": cos_t,
                "sin": sin_t,
                "masks": masks,
            }
        )

    import os

    trace = bool(os.environ.get("KERNEL_TRACE"))
    kw = {}
    if trace and os.environ.get("KERNEL_TRACE_DIR"):
        kw["tmpdir"] = os.environ["KERNEL_TRACE_DIR"]
    res = run_bass_kernel_spmd(nc, in_maps, list(range(n_cores)), trace=trace, **kw)
    global LAST_RESULT
    LAST_RESULT = res

    outs = [res.results[c]["outp"].astype(np.float32) for c in range(n_cores)]
    out = np.stack(
        [sum(outs[b * groups + g] for g in range(groups)) for b in range(B)],
        axis=0,
    )
    return out.astype(np.float32)

